# revision 12
# baseline (speedup 1.0000x reference)
"""ConcatSquashLinear + channel self-attention kernel for Trainium2 (8 NeuronCores).

Reference computation (per batch b; B=32, N=2048, Din=Dout=512, Dctx=256):
    gate = sigmoid(ctx @ W_gate.T + b_gate)            [1, Dout]
    bias = ctx @ W_bias.T                              [1, Dout]
    k    = ctx @ W_k.T                                 [1, Din]
    E    = outer(k, k)                                 [Din, Din] (symmetric)
    A    = softmax(E, axis=-1)                         row softmax
    A2   = A / (1e-9 + A.sum(axis=0))                  column renorm
    out  = ((x + x @ A2) @ W_layer.T) * gate + b_layer * gate + bias

Algebraic restructuring (per batch):
    r_row[i] = 1 / sum_j exp(E[i,j])
    colsum[j] = sum_i exp(E[i,j]) * r_row[i]
    r_col[j] = 1 / (1e-9 + colsum[j])
    W2       = diag(r_col) @ W_layer.T
    Mtot     = W_layer.T + diag(r_row) @ (expE @ W2)   [Din, Dout]
    c[o]     = b_layer[o] * gate[o] + bias[o]
    out[n,o] = sum_i x[n,i] Mtot[i,o];  out = out * gate + c

v4 strategy (baseline 190us -> v2 119us -> v3 108.6us -> v4):
  * x host-pre-transposed/cast: xT [BPC, Din, N] bf16; output computed
    transposed (Mtot blocks PE-stationary), written bf16.
  * gate/c folded into the PSUM drain as a per-partition affine on
    scalar/vector; sigmoid via exp so one ACT table serves everything.
  * expE/W2/r_row in fp8e4 (scaled x256/x8) -> P and colsum matmuls run
    DoubleRow; touches only the x_r correction (~4% of output).
  * All weights bf16. DMA issue is spread across engines (weights on
    the scalar queue, x batch 0/1 on gpsimd, 2/3 on sync; outputs on
    gpsimd) and small weights are merged, so the k->energy->exp->...
    ->Mtot chain for batch 0 starts as early as the queue preamble
    allows instead of waiting ~13us behind a single serial DMA queue.
  * Mtot assembled with one fused scalar_tensor_tensor per chunk.

Sharding: data-parallel over batch, 4 batches per core, weights replicated.
"""

import sys

import numpy as np

try:
    import concourse.bass as bass  # noqa: F401
except ImportError:  # pragma: no cover - path fallback for fresh dirs
    for _p in ("/opt/trn_rl_repo", "/root/.axon_site/_ro/trn_rl_repo"):
        if _p not in sys.path:
            sys.path.append(_p)
    import concourse.bass as bass  # noqa: F401

import ml_dtypes
import concourse.tile as tile
from concourse import bacc, mybir
from concourse.bass_utils import run_bass_kernel_spmd

B, N, DIN, DOUT, DCTX = 32, 2048, 512, 512, 256
NCORES = 8
BPC = B // NCORES      # batches per core
IC = DIN // 128        # 4 channel chunks (contraction dim)
TC = DOUT // 128       # 4 output-channel chunks
CC = DCTX // 128       # 2 ctx chunks
NB = N // 512          # 4 point-blocks of 512 (one PSUM bank wide)

F32 = mybir.dt.float32
F32R = mybir.dt.float32r
BF16 = mybir.dt.bfloat16
FP8 = mybir.dt.float8e4
AF = mybir.ActivationFunctionType
ALU = mybir.AluOpType
DROW = mybir.MatmulPerfMode.DoubleRow

# fp8 scale factors (powers of two; folded back out downstream)
RR_S = 256.0    # r_row ~ 1/600 -> x256 keeps it normal in e4m3
W2_S = 8.0      # W2 ~ +-0.05  -> x8

OUT_NAME = "outT"


def build_program(mode="bf16"):
    assert mode == "bf16", "v4 kernel only implements the bf16 pipeline"
    nc = bacc.Bacc("TRN2", target_bir_lowering=False, debug=False)

    xT_d = nc.dram_tensor("xT", [BPC, DIN, N], BF16, kind="ExternalInput")
    ctxT_d = nc.dram_tensor("ctxT", [DCTX, BPC], BF16, kind="ExternalInput")
    wkT_d = nc.dram_tensor("wkT", [DCTX, DIN], BF16, kind="ExternalInput")
    wgb_d = nc.dram_tensor("wgb", [2, DCTX, DOUT], BF16, kind="ExternalInput")
    wlT_d = nc.dram_tensor("wlT", [DIN, DOUT], BF16, kind="ExternalInput")
    bgbl_d = nc.dram_tensor("bgbl", [DOUT, 2], F32, kind="ExternalInput")
    out_d = nc.dram_tensor(OUT_NAME, [BPC, DOUT, N], BF16, kind="ExternalOutput")

    with tile.TileContext(nc) as tc:
        with (
            tc.tile_pool(name="wpool", bufs=1) as wpool,
            tc.tile_pool(name="mpool", bufs=2) as mpool,
            tc.tile_pool(name="spool", bufs=2) as spool,
            tc.tile_pool(name="opool", bufs=3) as opool,
            tc.tile_pool(name="psum", bufs=1, space="PSUM") as psum,
        ):
            # ---- all inputs on the sync queue: it is the only DGE path that
            # sprays packets across all 16 DMA engines (full ~358GB/s) ----
            ctx_sb = wpool.tile([128, CC, BPC], BF16)
            nc.sync.dma_start(out=ctx_sb, in_=ctxT_d.rearrange("(c p) b -> p c b", p=128))
            wk_sb = wpool.tile([128, CC, DIN], BF16)
            nc.sync.dma_start(out=wk_sb, in_=wkT_d.rearrange("(c p) i -> p c i", p=128))
            wl_sb = wpool.tile([128, IC, DOUT], BF16)
            nc.sync.dma_start(out=wl_sb, in_=wlT_d.rearrange("(c p) o -> p c o", p=128))
            wgb_sb = wpool.tile([128, 2, CC, DOUT], BF16)
            nc.sync.dma_start(out=wgb_sb, in_=wgb_d.rearrange("g (c p) o -> p g c o", p=128))
            bgbl_sb = wpool.tile([128, TC, 2], F32)
            nc.sync.dma_start(out=bgbl_sb, in_=bgbl_d.rearrange("(t p) g -> p t g", p=128))
            wg_sb = wgb_sb[:, 0]
            wb_sb = wgb_sb[:, 1]
            bgn_sb = bgbl_sb[:, :, 0:1]
            bl_sb = bgbl_sb[:, :, 1:2]

            xall = wpool.tile([128, BPC, IC, N], BF16)
            for b in range(BPC):
                nc.sync.dma_start(out=xall[:, b],
                                  in_=xT_d[b].rearrange("(c p) n -> p c n", p=128))

            # ---------------- hyper-network pieces ----------------
            k_sb = wpool.tile([1, BPC, DIN], F32R)

            def emit_k(b):
                kp = psum.tile([128, DIN], F32, tag="eng", bufs=2, name=f"kp{b}")
                for c in range(CC):
                    nc.tensor.matmul(kp[:1, :], ctx_sb[:, c, b:b + 1],
                                     wk_sb[:, c, :],
                                     start=(c == 0), stop=(c == CC - 1))
                nc.vector.tensor_copy(k_sb[:, b, :], kp[:1, :])

            # gate/c in column layout [o-part, t, b]; their PSUM lives in the
            # "out" tag rotation (free until the first main drains ~19us)
            gcol_ps = psum.tile([128, 40], F32, tag="out", bufs=3)
            bcol_ps = psum.tile([128, 40], F32, tag="out", bufs=3)
            egate = wpool.tile([128, TC, BPC], F32)
            gate_col = wpool.tile([128, TC, BPC], F32)
            tmp1 = wpool.tile([128, TC, BPC], F32)
            c_col = wpool.tile([128, TC, BPC], F32)
            gbl = wpool.tile([128, TC, BPC], F32)

            def emit_gate():
                for t in range(TC):
                    gp = gcol_ps[:, 4 * t:4 * (t + 1)]
                    for c in range(CC):
                        nc.tensor.matmul(gp, wg_sb[:, c, 128 * t:128 * (t + 1)],
                                         ctx_sb[:, c, :],
                                         start=(c == 0), stop=(c == CC - 1))
                    # exp(-(z + b_gate)) ; bgbl[...,0] = -b_gate
                    nc.scalar.activation(egate[:, t, :], gp, AF.Exp,
                                         bias=bgn_sb[:, t, :], scale=-1.0)
                nc.vector.tensor_scalar_add(tmp1, egate, 1.0)
                nc.vector.reciprocal(gate_col, tmp1)  # sigmoid done

            def emit_cbias():
                for t in range(TC):
                    bp = bcol_ps[:, 4 * t:4 * (t + 1)]
                    for c in range(CC):
                        nc.tensor.matmul(bp, wb_sb[:, c, 128 * t:128 * (t + 1)],
                                         ctx_sb[:, c, :],
                                         start=(c == 0), stop=(c == CC - 1))
                    nc.vector.tensor_scalar(gbl[:, t, :], gate_col[:, t, :],
                                            bl_sb[:, t, :], None, ALU.mult)
                    nc.vector.tensor_add(c_col[:, t, :], gbl[:, t, :], bp)

            # ---------------- per-batch attention precompute ----------------
            state = {}

            def pre_energy(b):
                expE = mpool.tile([128, IC, DIN], FP8, tag="expE", bufs=4,
                                  name=f"expE{b}")
                rs = spool.tile([128, IC], F32, tag="rs", name=f"rs{b}")
                for d in range(IC):
                    eng_ps = psum.tile([128, DIN], F32, tag="eng", bufs=2,
                                       name=f"eng{b}{d}")
                    nc.tensor.matmul(eng_ps, k_sb[:, b, 128 * d:128 * (d + 1)],
                                     k_sb[:, b, :], start=True, stop=True)
                    nc.scalar.activation(expE[:, d, :], eng_ps, AF.Exp,
                                         accum_out=rs[:, d:d + 1])
                rrow_f = spool.tile([128, IC], F32, tag="rrowf", name=f"rrf{b}")
                nc.vector.reciprocal(rrow_f, rs)
                # r_row * 256 in fp8 (duplicated: matmuls want >=2 cols)
                rrow8 = spool.tile([128, IC, 2], FP8, tag="rrow", name=f"rr{b}")
                nc.vector.tensor_scalar(rrow8[:, :, 0], rrow_f, RR_S, None, ALU.mult)
                nc.vector.tensor_scalar(rrow8[:, :, 1], rrow_f, RR_S, None, ALU.mult)
                # r_row / W2_S for the Mtot rescale
                rrow_s = spool.tile([128, IC], F32, tag="rrows", name=f"rrs{b}")
                nc.vector.tensor_scalar(rrow_s, rrow_f, 1.0 / W2_S, None, ALU.mult)
                state[b] = dict(expE=expE, rrow8=rrow8, rrow_s=rrow_s)

            def pre_cs(b):
                st = state[b]
                expE, rrow8 = st["expE"], st["rrow8"]
                cs_ps = psum.tile([128, IC, 2], F32, tag="small", bufs=1,
                                  name=f"cs{b}")
                for d in range(IC):
                    for j in range(IC // 2):
                        nc.tensor.matmul(cs_ps[:, d, :],
                                         expE[:, 2 * j:2 * j + 2, 128 * d:128 * (d + 1)],
                                         rrow8[:, 2 * j:2 * j + 2, :],
                                         perf_mode=DROW,
                                         start=(j == 0), stop=(j == IC // 2 - 1))
                # colsum = cs/RR_S ; rcol_s = W2_S / (colsum + 1e-9)
                cst = spool.tile([128, IC], F32, tag="cst", name=f"cst{b}")
                nc.vector.tensor_scalar(cst, cs_ps[:, :, 0], 1.0 / RR_S, 1e-9,
                                        ALU.mult, ALU.add)
                rcol = spool.tile([128, IC], F32, tag="rcol", name=f"rc{b}")
                nc.vector.reciprocal(rcol, cst)
                rcol_s = spool.tile([128, IC], F32, tag="rcols", name=f"rcs{b}")
                nc.vector.tensor_scalar(rcol_s, rcol, W2_S, None, ALU.mult)
                wg2 = mpool.tile([128, IC, DOUT], FP8, tag="wg2", bufs=4,
                                 name=f"wg2{b}")
                for d in range(IC):
                    if d % 2 == 0:
                        nc.scalar.activation(wg2[:, d, :], wl_sb[:, d, :],
                                             AF.Copy, scale=rcol_s[:, d:d + 1])
                    else:
                        nc.vector.tensor_scalar(wg2[:, d, :], wl_sb[:, d, :],
                                                rcol_s[:, d:d + 1], None, ALU.mult)
                st["wg2"] = wg2

            def pre_P(b):
                st = state[b]
                expE, wg2, rrow_s = st["expE"], st["wg2"], st["rrow_s"]
                mtot = [mpool.tile([128, DOUT], BF16, tag=f"mtot{d}", bufs=4,
                                   name=f"mtot{b}_{d}") for d in range(IC)]
                for d in range(IC):
                    p_ps = psum.tile([128, DOUT], F32, tag="p", bufs=2,
                                     name=f"pps{b}{d}")
                    for j in range(IC // 2):
                        nc.tensor.matmul(p_ps,
                                         expE[:, 2 * j:2 * j + 2, 128 * d:128 * (d + 1)],
                                         wg2[:, 2 * j:2 * j + 2, :],
                                         perf_mode=DROW,
                                         start=(j == 0), stop=(j == IC // 2 - 1))
                    # Mtot = (P * r_row/W2_S) + W_layer.T in one fused op
                    nc.vector.scalar_tensor_tensor(mtot[d], p_ps,
                                                   rrow_s[:, d:d + 1],
                                                   wl_sb[:, d, :],
                                                   ALU.mult, ALU.add)
                st["mtot"] = mtot

            # ---------------- main matmuls, one t-tile at a time ----------------
            def main_t(b, t):
                mtot = state[b]["mtot"]
                ostage = opool.tile([128, N], BF16, tag="ost", bufs=3,
                                    name=f"ost{b}{t}")
                for nb in range(NB):
                    ops = psum.tile([128, 512], F32, tag="out", bufs=3,
                                    name=f"ops{b}{t}{nb}")
                    for i in range(IC):
                        nc.tensor.matmul(ops,
                                         mtot[i][:, 128 * t:128 * (t + 1)],
                                         xall[:, b, i, 512 * nb:512 * (nb + 1)],
                                         start=(i == 0), stop=(i == IC - 1))
                    gs = gate_col[:, t, b:b + 1]
                    cc = c_col[:, t, b:b + 1]
                    dst = ostage[:, 512 * nb:512 * (nb + 1)]
                    if nb % 2 == 0:
                        nc.scalar.activation(dst, ops, AF.Identity,
                                             bias=cc, scale=gs)
                    else:
                        nc.vector.tensor_scalar(dst, ops, gs, cc,
                                                ALU.mult, ALU.add)
                        # DMA out each half as soon as it is drained
                        nc.gpsimd.dma_start(
                            out=out_d[b, 128 * t:128 * (t + 1),
                                      512 * (nb - 1):512 * (nb + 1)],
                            in_=ostage[:, 512 * (nb - 1):512 * (nb + 1)])

            # ---------------- schedule ----------------
            # ALL precompute in the prologue: its tensor work fills the PE
            # while the batch-0 dependency chain settles, and the main
            # matmul stream then runs back-to-back with no interleaved
            # dependencies.
            emit_k(0)
            pre_energy(0)
            for b in range(1, BPC):
                emit_k(b)
            emit_gate()
            pre_cs(0)
            emit_cbias()
            pre_P(0)
            for b in range(1, BPC):
                pre_energy(b)
                pre_cs(b)
                pre_P(b)
            for b in range(BPC):
                for t in range(TC):
                    main_t(b, t)

    return nc


def prep_inputs(ctx, x, W_layer, b_layer, W_bias, W_gate, b_gate, W_k):
    """Host-side layout prep + per-core sharding. Returns in_maps for 8 cores."""
    f = np.float32
    bf = ml_dtypes.bfloat16
    wkT = np.ascontiguousarray(np.asarray(W_k).T, dtype=bf)       # [DCTX, DIN]
    wgb = np.ascontiguousarray(
        np.stack([np.asarray(W_gate).T, np.asarray(W_bias).T]), dtype=bf)
    wlT = np.ascontiguousarray(np.asarray(W_layer).T, dtype=bf)   # [DIN, DOUT]
    bgbl = np.ascontiguousarray(
        np.stack([-np.asarray(b_gate), np.asarray(b_layer)], axis=1), dtype=f)
    xbf = np.asarray(x).astype(bf)                                # [B, N, DIN]
    ctx = np.asarray(ctx)
    in_maps = []
    for core in range(NCORES):
        s = slice(core * BPC, (core + 1) * BPC)
        in_maps.append({
            "xT": np.ascontiguousarray(xbf[s].transpose(0, 2, 1)),  # [BPC, DIN, N]
            "ctxT": np.ascontiguousarray(ctx[s, 0, :].T.astype(bf)),
            "wkT": wkT, "wgb": wgb, "wlT": wlT, "bgbl": bgbl,
        })
    return in_maps


def postprocess_core(arr):
    """[BPC, DOUT, N] bf16 -> [BPC, N, DOUT] f32."""
    return np.ascontiguousarray(
        np.asarray(arr).astype(np.float32).transpose(0, 2, 1))


def run(inputs, mode="bf16", trace=False, **kw):
    nc = build_program(mode=mode)
    nc.finalize()
    in_maps = prep_inputs(**inputs)
    res = run_bass_kernel_spmd(nc, in_maps, list(range(NCORES)), trace=trace, **kw)
    out = np.concatenate(
        [postprocess_core(res.results[i][OUT_NAME]) for i in range(NCORES)], axis=0)
    return out.astype(np.float32), res


def kernel(**inputs):
    out, _ = run(inputs)
    return out
